# revision 23
# baseline (speedup 1.0000x reference)
"""4D Conv-MLP (conv3^4 -> ReLU -> conv3^4) on 8 Trainium2 NeuronCores.

Sharding: core = b*4 + j  (batch b in {0,1}, H-slab j in {0..3}, 8 output rows
each). conv1 computes 9 h rows (own 8 + the halo row on the pair-external
side, recomputed from the host-provided x halo); the pair-internal halo row is
exchanged with the LNC sibling (cores 2k/2k+1 share their DRAM "Shared"
address space) via plain dynamic DMAs — no collective in the steady state.
Odd-j cores operate on H-flipped data (host flips x rows and kv-reverses the
weights) so the program is exactly SPMD: every core sends local row 8 and
receives its local row 9. Exchange readbacks are scheduled a full conv1
t-phase (or, for t=3, one conv2 t-phase) after the matching writes, so
cross-core ordering holds with >60us margin against the ~us launch skew of
sibling cores; kernel() additionally reruns the NEFF until two consecutive
bitwise-identical finite outputs agree, absorbing rare skewed first launches
(correct runs are deterministic, and after any run the shared buffers already
hold correct halo data).

On-chip algorithm (implicit GEMM over the 81 taps, fp16 operands, fp32 PSUM):
  - x lives channel-on-partition as zero-padded flat planes per t ([16 D][11 H]
    [34 W], +1 lead pad), in two SBUF tiles of two shifted copies each:
    tileA = (x, x+1) and tileB = (x+2, x+36), so most K=128 matmuls contract
    two taps at once.
  - conv1: per (t, d): N=306 matmuls; each valid (kt, ku) block = 4 K=128
    pairs + 1 K=64 single (optimal for a 3x3 (kv, kw) grid with shift deltas
    {1, 34}); all-zero T/D edge taps are skipped; ReLU+bias on the Scalar
    engine writes fp16 h rows 0..8; out-of-image row 0 zeroed via mask m0.
  - conv2: N=512 runs over d-pairs (N=256 at D edges, pad taps skipped);
    taps alternate PE column groups via tile_position (0,0)/(0,64) so two
    M=64 matmuls run concurrently; halves summed + bias on Scalar/DVE.
  Known pitfall baked into the structure: tile_size transitions stall the
  LDWEIGHTS pipeline, so K=64 singles are batched at the end of each conv1
  accumulation chain.
"""

import numpy as np

B, C_IN, C_HID, C_OUT = 2, 64, 128, 64
T, D, H, W = 4, 16, 32, 32
NCORES, NJ = 8, 4
SH = H // NJ          # 8 out rows per slab
HR = SH + 1           # 9 computed h rows (own 8 + pair-external halo)
XH = HR + 2           # 11 x rows per slab
HHH = SH + 2          # 10 h rows per slab (9 computed + 1 exchanged)
XROW = 34             # padded W
XDP = XH * XROW       # 374
XP = 1 + D * XDP + 7  # x plane size = 5992
HD, HW_ = 18, 34
N1 = HR * XROW        # conv1 run = 306
N2 = 512              # conv2 run (2 d-rows)

_cache = {}


def _t_taps(t):
    return [kt for kt in range(3) if 0 <= t + kt - 1 < T]


def _g27(kt, ku, kv):
    return (kt * 3 + ku) * 3 + kv


def _g81(kt, ku, kv, kw):
    return ((kt * 3 + ku) * 3 + kv) * 3 + kw


def _make_host_arrays(x, w1, b1, w2, b2):
    x = np.asarray(x, np.float32)
    Xs, M0s = [], []
    for core in range(NCORES):
        b, j = divmod(core, NJ)
        p = j % 2
        # x local row r maps to global row (8j-2+r) unflipped, (8j+9-r)
        # flipped; out-of-image rows zero
        if p == 0:
            rows = [8 * j - 2 + r for r in range(XH)]
        else:
            rows = [8 * j + 9 - r for r in range(XH)]
        slab = np.zeros((C_IN, T, D, XH, W), np.float32)
        for r, g in enumerate(rows):
            if 0 <= g < H:
                slab[:, :, :, r, :] = x[b, :, :, :, g, :]
        plane = np.zeros((C_IN, T, D, XH, XROW), np.float32)
        plane[:, :, :, :, 1:33] = slab
        flat = plane.reshape(C_IN, T, D * XDP)
        X = np.zeros((C_IN, T, XP), np.float16)
        X[:, :, 1:1 + D * XDP] = flat
        Xs.append(X)
        M0s.append(np.full((128, 1), 0.0 if j in (0, NJ - 1) else 1.0,
                           np.float32))

    w1 = np.asarray(w1, np.float32)
    w2 = np.asarray(w2, np.float32)
    W1Ps, W1PBs, W1Ss, W2s = [], [], [], []
    for pr in range(2):
        def kvm(kv):
            return kv if pr == 0 else 2 - kv
        W1P = np.zeros((128, 27, 128), np.float16)   # tileA: (kv,0)+(kv,1)
        W1PB = np.zeros((128, 9, 128), np.float16)   # tileB: (0,2)+(1,2)
        W1S = np.zeros((128, 9, 128), np.float16)    # single: (2,2)
        for kt in range(3):
            for ku in range(3):
                g9 = kt * 3 + ku
                W1PB[:64, g9, :] = w1[:, :, kt, ku, kvm(0), 2].T
                W1PB[64:, g9, :] = w1[:, :, kt, ku, kvm(1), 2].T
                W1S[:64, g9, :] = w1[:, :, kt, ku, kvm(2), 2].T
                for kv in range(3):
                    g = _g27(kt, ku, kv)
                    W1P[:64, g, :] = w1[:, :, kt, ku, kvm(kv), 0].T
                    W1P[64:, g, :] = w1[:, :, kt, ku, kvm(kv), 1].T
        W2 = np.zeros((128, 81, 64), np.float16)
        for kt in range(3):
            for ku in range(3):
                for kv in range(3):
                    for kw in range(3):
                        gi = _g81(kt, ku, kv, kw)
                        W2[:, gi, :] = w2[:, :, kt, ku, kvm(kv), kw].T
        W1Ps.append(W1P.reshape(128, 27 * 128))
        W1PBs.append(W1PB.reshape(128, 9 * 128))
        W1Ss.append(W1S.reshape(128, 9 * 128))
        W2s.append(W2.reshape(128, 81 * 64))
    return dict(X=Xs, M0=M0s,
                W1P=W1Ps, W1PB=W1PBs, W1S=W1Ss, W2=W2s,
                B1=np.asarray(b1, np.float32).reshape(128, 1),
                B2=np.asarray(b2, np.float32).reshape(64, 1))


def _in_maps(hostd):
    maps = []
    for core in range(NCORES):
        j = core % NJ
        pr = j % 2
        maps.append({
            "x": hostd["X"][core], "m0": hostd["M0"][core],
            "w1p": hostd["W1P"][pr], "w1pb": hostd["W1PB"][pr],
            "w1s": hostd["W1S"][pr], "w2": hostd["W2"][pr],
            "b1": hostd["B1"], "b2": hostd["B2"],
        })
    return maps


def _build_module():
    import concourse.bass as bass
    import concourse.tile as tile
    from concourse import bacc, mybir

    fp16 = mybir.dt.float16
    fp32 = mybir.dt.float32

    nc = bacc.Bacc("TRN2", target_bir_lowering=False, debug=False,
                   num_devices=NCORES)
    x_d = nc.dram_tensor("x", [64, T, XP], fp16, kind="ExternalInput")
    w1p_d = nc.dram_tensor("w1p", [128, 27 * 128], fp16, kind="ExternalInput")
    w1pb_d = nc.dram_tensor("w1pb", [128, 9 * 128], fp16, kind="ExternalInput")
    w1s_d = nc.dram_tensor("w1s", [128, 9 * 128], fp16, kind="ExternalInput")
    w2_d = nc.dram_tensor("w2", [128, 81 * 64], fp16, kind="ExternalInput")
    b1_d = nc.dram_tensor("b1", [128, 1], fp32, kind="ExternalInput")
    b2_d = nc.dram_tensor("b2", [64, 1], fp32, kind="ExternalInput")
    m0_d = nc.dram_tensor("m0", [128, 1], fp32, kind="ExternalInput")
    y_d = nc.dram_tensor("y", [64, T, D * SH * W], fp32, kind="ExternalOutput")

    with tile.TileContext(nc) as tc:
        with (
            tc.tile_pool(name="xw", bufs=1) as xw,
            tc.tile_pool(name="hp", bufs=1) as hpool,
            tc.tile_pool(name="st", bufs=4) as stp,
            tc.tile_pool(name="dr", bufs=1, space="DRAM") as dr,
            tc.tile_pool(name="p1", bufs=6, space="PSUM") as p1,
            tc.tile_pool(name="p2", bufs=2, space="PSUM") as p2,
        ):
            w1p = xw.tile([128, 27, 128], fp16)
            nc.gpsimd.dma_start(w1p[:, 9:27, :], w1p_d.ap()[:, 9 * 128:])

            pid = nc.partition_id()
            pp = pid % 2
            b1 = xw.tile([128, 1], fp32)
            nc.sync.dma_start(b1[:, :], b1_d.ap())

            # tileA = (x, x+1), tileB = (x+2, x+36): shifted copies so each
            # K=128 matmul contracts two taps; quarter-chunked, first chunks
            # DMAd first so conv1 can start early
            xa = xw.tile([128, T, XP], fp16)
            xb = xw.tile([128, T, XP], fp16)
            # the shifted-copy DMAs leave the last s columns unwritten; the
            # K=128 singles read xb's upper half up to XP-8, so zero the tail
            for t in range(T):
                nc.vector.memset(xb[64:128, t, XP - 40:XP], 0.0)
            qs = [0, XP // 4, XP // 2, 3 * XP // 4, XP]

            def xchunk(t, ci):
                lo, hi = qs[ci], qs[ci + 1]
                for tdst, p0, s in ((xa, 0, 0), (xa, 64, 1),
                                    (xb, 0, 2), (xb, 64, 36)):
                    he = min(hi, XP - s)
                    nc.sync.dma_start(tdst[p0:p0 + 64, t, lo:he],
                                      x_d.ap()[:, t, lo + s:he + s])

            # conv1 t=0 needs planes 0,1 and the tileB weights first
            xchunk(0, 0)
            xchunk(1, 0)
            w1pb = xw.tile([128, 9, 128], fp16)
            nc.gpsimd.dma_start(w1pb[:, :, :], w1pb_d.ap())
            w1s = xw.tile([128, 9, 128], fp16)
            nc.gpsimd.dma_start(w1s[:, :, :], w1s_d.ap())
            nc.gpsimd.dma_start(w1p[:, 0:9, :], w1p_d.ap()[:, 0:9 * 128])
            for t, ci in ((0, 1), (1, 1), (2, 0), (3, 0), (0, 2), (1, 2),
                          (0, 3), (1, 3), (2, 1), (3, 1), (2, 2), (3, 2),
                          (2, 3), (3, 3)):
                xchunk(t, ci)

            w2 = xw.tile([128, 81, 64], fp16)
            nc.sync.dma_start(w2[:, :, :], w2_d.ap())
            b2 = xw.tile([64, 1], fp32)
            nc.sync.dma_start(b2[:, :], b2_d.ap())
            m0 = xw.tile([128, 1], fp32)
            nc.sync.dma_start(m0[:, :], m0_d.ap())

            ht = hpool.tile([128, T, HD, HHH, HW_], fp16)
            for t in range(T):
                nc.vector.memset(ht[:, t, 1:17, :, 0:1], 0.0)
                nc.vector.memset(ht[:, t, 1:17, :, 33:34], 0.0)

            # pair-shared halo exchange buffers (one per t, 2 slots)
            sx = [dr.tile([2, 128, D, W], fp16, name=f"sx{t}",
                          addr_space="Shared")
                  for t in range(T)]

            def halo_read(t):
                nc.gpsimd.dma_start(ht[:, t, 1:17, HHH - 1, 1:33],
                                    sx[t][bass.ds(1 - pp, 1), :, :, :])

            # ---- conv1 + per-t halo exchange ----
            # per valid (kt, ku) block: 4 K=128 pairs + 1 K=64 single:
            #   tileA pairs at q=Bq+kv*34 cover (kv,kw=0)+(kv,kw=1)
            #   tileB pair  at q=Bq       covers (0,2)+(1,2)
            #   tileB-top single at q=Bq+68 covers (2,2)
            for t in range(T):
                for d in range(D):
                    blocks = [(kt, ku) for kt in _t_taps(t)
                              for ku in range(3) if 0 <= d + ku - 1 < D]
                    ps = p1.tile([128, HR, XROW], fp32)
                    # all K=128 matmuls first, then all K=64 singles, so the
                    # PE sees only one tile_size transition per run (tile
                    # switches stall the LDWEIGHTS pipeline)
                    i = 0
                    for kt, ku in blocks:
                        tp = t + kt - 1
                        bq = (d + ku - 1) * XDP
                        for kv in range(3):
                            nc.tensor.matmul(
                                ps[:, :, :], w1p[:, _g27(kt, ku, kv), :],
                                xa[:, tp, bq + kv * XROW:bq + kv * XROW + N1],
                                start=(i == 0), stop=False)
                            i += 1
                        nc.tensor.matmul(
                            ps[:, :, :], w1pb[:, kt * 3 + ku, :],
                            xb[:, tp, bq:bq + N1],
                            start=False, stop=False)
                        i += 1
                    for i, (kt, ku) in enumerate(blocks):
                        tp = t + kt - 1
                        bq = (d + ku - 1) * XDP
                        nc.tensor.matmul(
                            ps[:, :, :], w1s[:, kt * 3 + ku, :],
                            xb[:, tp, bq + 68:bq + 68 + N1],
                            start=False, stop=(i == len(blocks) - 1))
                    nc.scalar.activation(
                        ht[:, t, d + 1, 0:HR, 1:33], ps[:, :, 1:33],
                        mybir.ActivationFunctionType.Relu, bias=b1[:, 0:1])
                # zero out-of-image halo row 0 (mask 0 only on edge cores)
                nc.vector.tensor_scalar_mul(
                    ht[:, t, 1:17, 0, 1:33], ht[:, t, 1:17, 0, 1:33],
                    m0[:, 0:1])
                # send own pair-edge row (local row 8) to the sibling slot
                nc.gpsimd.dma_start(sx[t][bass.ds(pp, 1), :, :, :],
                                    ht[:, t, 1:17, SH, 1:33])
                # read sibling halo of t-1: a full conv1 t-phase (~90us)
                # after the matching write on the other side
                if t >= 1:
                    halo_read(t - 1)

            # ---- conv2 ----
            # runs: edge d=0 and d=15 alone (N=256, zero-pad taps skipped),
            # interior d as 7 pairs (N=512). Taps alternate between PE column
            # groups (psum partitions 0:64 / 64:128) so adjacent matmuls run
            # concurrently; halves summed via Scalar+DVE into the stage tile.
            runs = [(0, 1)] + [(d0, 2) for d0 in range(1, 15, 2)] + [(15, 1)]
            scrapd = dr.tile([64, 32], fp32, name="scrapd")
            for t in range(T):
                for d0, nd in runs:
                    taps = [(kt, ku, kv, kw) for kt in _t_taps(t)
                            for ku in range(3) if 0 < d0 + ku < 17 or nd == 2
                            for kv in range(3) for kw in range(3)]
                    nn = nd * SH * W
                    lo = taps[0::2]
                    hi = taps[1::2]
                    ps = p2.tile([128, N2], fp32)
                    for i in range(len(lo)):
                        for half, base, tp_pos in ((lo, 0, (0, 0)),
                                                   (hi, 64, (0, 64))):
                            if i >= len(half):
                                continue
                            kt, ku, kv, kw = half[i]
                            gi = _g81(kt, ku, kv, kw)
                            rhs = ht[:, t + kt - 1, d0 + ku:d0 + ku + nd,
                                     kv:kv + SH, kw:kw + W]
                            nc.tensor.matmul(
                                ps[base:base + 64, 0:nn], w2[:, gi, :], rhs,
                                start=(i == 0), stop=(i == len(half) - 1),
                                tile_position=tp_pos)
                    st = stp.tile([64, N2], fp32)
                    nc.scalar.activation(
                        st[:, 0:nn], ps[64:128, 0:nn],
                        mybir.ActivationFunctionType.Identity, bias=b2[:, 0:1])
                    nc.vector.tensor_add(st[:, 0:nn], st[:, 0:nn],
                                         ps[0:64, 0:nn])
                    nc.sync.dma_start(
                        y_d.ap()[:, t, d0 * SH * W:d0 * SH * W + nn],
                        st[:, 0:nn])
                if t == 0:
                    # gate the last halo readback on conv2 t=0 having
                    # drained, so it runs a conv2 t-phase after the
                    # sibling's t=3 write (conv2 t>=2 is what consumes it)
                    nc.gpsimd.dma_start(scrapd[:, :], st[:, 0:32])
                    halo_read(T - 1)
    nc.compile()
    return nc


def kernel(x, w1, b1, w2, b2):
    from concourse.bass_utils import run_bass_kernel_spmd

    hostd = _make_host_arrays(x, w1, b1, w2, b2)
    if "nc" not in _cache:
        _cache["nc"] = _build_module()
    nc = _cache["nc"]

    in_maps = _in_maps(hostd)

    def one_run():
        res = run_bass_kernel_spmd(nc, in_maps, core_ids=list(range(NCORES)))
        y = np.zeros((B, C_OUT, T, D, H, W), np.float32)
        for core in range(NCORES):
            b, j = divmod(core, NJ)
            yc = res.results[core]["y"].reshape(C_OUT, T, D, SH, W)
            if j % 2 == 0:
                y[b, :, :, :, SH * j:SH * (j + 1), :] = yc
            else:
                y[b, :, :, :, SH * j:SH * (j + 1), :] = yc[:, :, :, ::-1, :]
        return y

    # the cross-core halo exchange relies on launch-aligned cores; a rare
    # skewed launch corrupts a run, so require two consecutive identical
    # finite outputs (correct runs are bitwise deterministic)
    prev = one_run()
    for _ in range(4):
        cur = one_run()
        if np.isfinite(cur).all() and np.array_equal(cur, prev):
            return cur
        prev = cur
    return prev


# revision 24
# speedup vs baseline: 1.0015x; 1.0015x over previous
"""4D Conv-MLP (conv3^4 -> ReLU -> conv3^4) on 8 Trainium2 NeuronCores.

Sharding: core = b*4 + j  (batch b in {0,1}, H-slab j in {0..3}, 8 output rows
each). conv1 computes 9 h rows (own 8 + the halo row on the pair-external
side, recomputed from the host-provided x halo); the pair-internal halo row is
exchanged with the LNC sibling (cores 2k/2k+1 share their DRAM "Shared"
address space) via plain dynamic DMAs — no collective in the steady state.
Odd-j cores operate on H-flipped data (host flips x rows and kv-reverses the
weights) so the program is exactly SPMD: every core sends local row 8 and
receives its local row 9. Exchange readbacks are scheduled a full conv1
t-phase (or, for t=3, one conv2 t-phase) after the matching writes, so
cross-core ordering holds with >60us margin against the ~us launch skew of
sibling cores; kernel() additionally reruns the NEFF until two consecutive
bitwise-identical finite outputs agree, absorbing rare skewed first launches
(correct runs are deterministic, and after any run the shared buffers already
hold correct halo data).

On-chip algorithm (implicit GEMM over the 81 taps, fp16 operands, fp32 PSUM):
  - x lives channel-on-partition as zero-padded flat planes per t ([16 D][11 H]
    [34 W], +1 lead pad), in two SBUF tiles of two shifted copies each:
    tileA = (x, x+1) and tileB = (x+2, x+36), so most K=128 matmuls contract
    two taps at once.
  - conv1: per (t, d): N=306 matmuls; each valid (kt, ku) block = 4 K=128
    pairs + 1 K=64 single (optimal for a 3x3 (kv, kw) grid with shift deltas
    {1, 34}); all-zero T/D edge taps are skipped; ReLU+bias on the Scalar
    engine writes fp16 h rows 0..8; out-of-image row 0 zeroed via mask m0.
  - conv2: N=512 runs over d-pairs (N=256 at D edges, pad taps skipped);
    taps alternate PE column groups via tile_position (0,0)/(0,64) so two
    M=64 matmuls run concurrently; halves summed + bias on Scalar/DVE.
  Known pitfall baked into the structure: tile_size transitions stall the
  LDWEIGHTS pipeline, so K=64 singles are batched at the end of each conv1
  accumulation chain.
"""

import numpy as np

B, C_IN, C_HID, C_OUT = 2, 64, 128, 64
T, D, H, W = 4, 16, 32, 32
NCORES, NJ = 8, 4
SH = H // NJ          # 8 out rows per slab
HR = SH + 1           # 9 computed h rows (own 8 + pair-external halo)
XH = HR + 2           # 11 x rows per slab
HHH = SH + 2          # 10 h rows per slab (9 computed + 1 exchanged)
XROW = 34             # padded W
XDP = XH * XROW       # 374
XP = 1 + D * XDP + 7  # x plane size = 5992
HD, HW_ = 18, 34
N1 = HR * XROW        # conv1 run = 306
N2 = 512              # conv2 run (2 d-rows)

_cache = {}


def _t_taps(t):
    return [kt for kt in range(3) if 0 <= t + kt - 1 < T]


def _g27(kt, ku, kv):
    return (kt * 3 + ku) * 3 + kv


def _g81(kt, ku, kv, kw):
    return ((kt * 3 + ku) * 3 + kv) * 3 + kw


def _make_host_arrays(x, w1, b1, w2, b2):
    x = np.asarray(x, np.float32)
    Xs, M0s = [], []
    for core in range(NCORES):
        b, j = divmod(core, NJ)
        p = j % 2
        # x local row r maps to global row (8j-2+r) unflipped, (8j+9-r)
        # flipped; out-of-image rows zero
        if p == 0:
            rows = [8 * j - 2 + r for r in range(XH)]
        else:
            rows = [8 * j + 9 - r for r in range(XH)]
        slab = np.zeros((C_IN, T, D, XH, W), np.float32)
        for r, g in enumerate(rows):
            if 0 <= g < H:
                slab[:, :, :, r, :] = x[b, :, :, :, g, :]
        plane = np.zeros((C_IN, T, D, XH, XROW), np.float32)
        plane[:, :, :, :, 1:33] = slab
        flat = plane.reshape(C_IN, T, D * XDP)
        X = np.zeros((C_IN, T, XP), np.float16)
        X[:, :, 1:1 + D * XDP] = flat
        Xs.append(X)
        M0s.append(np.full((128, 1), 0.0 if j in (0, NJ - 1) else 1.0,
                           np.float32))

    w1 = np.asarray(w1, np.float32)
    w2 = np.asarray(w2, np.float32)
    W1Ps, W1PBs, W1Ss, W2s = [], [], [], []
    for pr in range(2):
        def kvm(kv):
            return kv if pr == 0 else 2 - kv
        W1P = np.zeros((128, 27, 128), np.float16)   # tileA: (kv,0)+(kv,1)
        W1PB = np.zeros((128, 9, 128), np.float16)   # tileB: (0,2)+(1,2)
        W1S = np.zeros((128, 9, 128), np.float16)    # single: (2,2)
        for kt in range(3):
            for ku in range(3):
                g9 = kt * 3 + ku
                W1PB[:64, g9, :] = w1[:, :, kt, ku, kvm(0), 2].T
                W1PB[64:, g9, :] = w1[:, :, kt, ku, kvm(1), 2].T
                W1S[:64, g9, :] = w1[:, :, kt, ku, kvm(2), 2].T
                for kv in range(3):
                    g = _g27(kt, ku, kv)
                    W1P[:64, g, :] = w1[:, :, kt, ku, kvm(kv), 0].T
                    W1P[64:, g, :] = w1[:, :, kt, ku, kvm(kv), 1].T
        W2 = np.zeros((128, 81, 64), np.float16)
        for kt in range(3):
            for ku in range(3):
                for kv in range(3):
                    for kw in range(3):
                        gi = _g81(kt, ku, kv, kw)
                        W2[:, gi, :] = w2[:, :, kt, ku, kvm(kv), kw].T
        W1Ps.append(W1P.reshape(128, 27 * 128))
        W1PBs.append(W1PB.reshape(128, 9 * 128))
        W1Ss.append(W1S.reshape(128, 9 * 128))
        W2s.append(W2.reshape(128, 81 * 64))
    return dict(X=Xs, M0=M0s,
                W1P=W1Ps, W1PB=W1PBs, W1S=W1Ss, W2=W2s,
                B1=np.asarray(b1, np.float32).reshape(128, 1),
                B2=np.asarray(b2, np.float32).reshape(64, 1))


def _in_maps(hostd):
    maps = []
    for core in range(NCORES):
        j = core % NJ
        pr = j % 2
        maps.append({
            "x": hostd["X"][core], "m0": hostd["M0"][core],
            "w1p": hostd["W1P"][pr], "w1pb": hostd["W1PB"][pr],
            "w1s": hostd["W1S"][pr], "w2": hostd["W2"][pr],
            "b1": hostd["B1"], "b2": hostd["B2"],
        })
    return maps


def _build_module():
    import concourse.bass as bass
    import concourse.tile as tile
    from concourse import bacc, mybir

    fp16 = mybir.dt.float16
    fp32 = mybir.dt.float32

    nc = bacc.Bacc("TRN2", target_bir_lowering=False, debug=False,
                   num_devices=NCORES)
    x_d = nc.dram_tensor("x", [64, T, XP], fp16, kind="ExternalInput")
    w1p_d = nc.dram_tensor("w1p", [128, 27 * 128], fp16, kind="ExternalInput")
    w1pb_d = nc.dram_tensor("w1pb", [128, 9 * 128], fp16, kind="ExternalInput")
    w1s_d = nc.dram_tensor("w1s", [128, 9 * 128], fp16, kind="ExternalInput")
    w2_d = nc.dram_tensor("w2", [128, 81 * 64], fp16, kind="ExternalInput")
    b1_d = nc.dram_tensor("b1", [128, 1], fp32, kind="ExternalInput")
    b2_d = nc.dram_tensor("b2", [64, 1], fp32, kind="ExternalInput")
    m0_d = nc.dram_tensor("m0", [128, 1], fp32, kind="ExternalInput")
    y_d = nc.dram_tensor("y", [64, T, D * SH * W], fp32, kind="ExternalOutput")

    with tile.TileContext(nc) as tc:
        with (
            tc.tile_pool(name="xw", bufs=1) as xw,
            tc.tile_pool(name="hp", bufs=1) as hpool,
            tc.tile_pool(name="st", bufs=4) as stp,
            tc.tile_pool(name="dr", bufs=1, space="DRAM") as dr,
            tc.tile_pool(name="p1", bufs=5, space="PSUM") as p1,
            tc.tile_pool(name="p2", bufs=3, space="PSUM") as p2,
        ):
            w1p = xw.tile([128, 27, 128], fp16)
            nc.gpsimd.dma_start(w1p[:, 9:27, :], w1p_d.ap()[:, 9 * 128:])

            pid = nc.partition_id()
            pp = pid % 2
            b1 = xw.tile([128, 1], fp32)
            nc.sync.dma_start(b1[:, :], b1_d.ap())

            # tileA = (x, x+1), tileB = (x+2, x+36): shifted copies so each
            # K=128 matmul contracts two taps; quarter-chunked, first chunks
            # DMAd first so conv1 can start early
            xa = xw.tile([128, T, XP], fp16)
            xb = xw.tile([128, T, XP], fp16)
            # the shifted-copy DMAs leave the last s columns unwritten; the
            # K=128 singles read xb's upper half up to XP-8, so zero the tail
            for t in range(T):
                nc.vector.memset(xb[64:128, t, XP - 40:XP], 0.0)
            qs = [0, XP // 4, XP // 2, 3 * XP // 4, XP]

            def xchunk(t, ci):
                lo, hi = qs[ci], qs[ci + 1]
                for tdst, p0, s in ((xa, 0, 0), (xa, 64, 1),
                                    (xb, 0, 2), (xb, 64, 36)):
                    he = min(hi, XP - s)
                    nc.sync.dma_start(tdst[p0:p0 + 64, t, lo:he],
                                      x_d.ap()[:, t, lo + s:he + s])

            # conv1 t=0 needs planes 0,1 and the tileB weights first
            xchunk(0, 0)
            xchunk(1, 0)
            w1pb = xw.tile([128, 9, 128], fp16)
            nc.gpsimd.dma_start(w1pb[:, :, :], w1pb_d.ap())
            w1s = xw.tile([128, 9, 128], fp16)
            nc.gpsimd.dma_start(w1s[:, :, :], w1s_d.ap())
            nc.gpsimd.dma_start(w1p[:, 0:9, :], w1p_d.ap()[:, 0:9 * 128])
            for t, ci in ((0, 1), (1, 1), (2, 0), (3, 0), (0, 2), (1, 2),
                          (0, 3), (1, 3), (2, 1), (3, 1), (2, 2), (3, 2),
                          (2, 3), (3, 3)):
                xchunk(t, ci)

            w2 = xw.tile([128, 81, 64], fp16)
            nc.sync.dma_start(w2[:, :, :], w2_d.ap())
            b2 = xw.tile([64, 1], fp32)
            nc.sync.dma_start(b2[:, :], b2_d.ap())
            m0 = xw.tile([128, 1], fp32)
            nc.sync.dma_start(m0[:, :], m0_d.ap())

            ht = hpool.tile([128, T, HD, HHH, HW_], fp16)
            for t in range(T):
                nc.vector.memset(ht[:, t, 1:17, :, 0:1], 0.0)
                nc.vector.memset(ht[:, t, 1:17, :, 33:34], 0.0)

            # pair-shared halo exchange buffers (one per t, 2 slots)
            sx = [dr.tile([2, 128, D, W], fp16, name=f"sx{t}",
                          addr_space="Shared")
                  for t in range(T)]

            def halo_read(t):
                nc.gpsimd.dma_start(ht[:, t, 1:17, HHH - 1, 1:33],
                                    sx[t][bass.ds(1 - pp, 1), :, :, :])

            # ---- conv1 + per-t halo exchange ----
            # per valid (kt, ku) block: 4 K=128 pairs + 1 K=64 single:
            #   tileA pairs at q=Bq+kv*34 cover (kv,kw=0)+(kv,kw=1)
            #   tileB pair  at q=Bq       covers (0,2)+(1,2)
            #   tileB-top single at q=Bq+68 covers (2,2)
            for t in range(T):
                for d in range(D):
                    blocks = [(kt, ku) for kt in _t_taps(t)
                              for ku in range(3) if 0 <= d + ku - 1 < D]
                    ps = p1.tile([128, HR, XROW], fp32)
                    # all K=128 matmuls first, then all K=64 singles, so the
                    # PE sees only one tile_size transition per run (tile
                    # switches stall the LDWEIGHTS pipeline)
                    i = 0
                    for kt, ku in blocks:
                        tp = t + kt - 1
                        bq = (d + ku - 1) * XDP
                        for kv in range(3):
                            nc.tensor.matmul(
                                ps[:, :, :], w1p[:, _g27(kt, ku, kv), :],
                                xa[:, tp, bq + kv * XROW:bq + kv * XROW + N1],
                                start=(i == 0), stop=False)
                            i += 1
                        nc.tensor.matmul(
                            ps[:, :, :], w1pb[:, kt * 3 + ku, :],
                            xb[:, tp, bq:bq + N1],
                            start=False, stop=False)
                        i += 1
                    for i, (kt, ku) in enumerate(blocks):
                        tp = t + kt - 1
                        bq = (d + ku - 1) * XDP
                        nc.tensor.matmul(
                            ps[:, :, :], w1s[:, kt * 3 + ku, :],
                            xb[:, tp, bq + 68:bq + 68 + N1],
                            start=False, stop=(i == len(blocks) - 1))
                    nc.scalar.activation(
                        ht[:, t, d + 1, 0:HR, 1:33], ps[:, :, 1:33],
                        mybir.ActivationFunctionType.Relu, bias=b1[:, 0:1])
                # zero out-of-image halo row 0 (mask 0 only on edge cores)
                nc.vector.tensor_scalar_mul(
                    ht[:, t, 1:17, 0, 1:33], ht[:, t, 1:17, 0, 1:33],
                    m0[:, 0:1])
                # send own pair-edge row (local row 8) to the sibling slot
                nc.gpsimd.dma_start(sx[t][bass.ds(pp, 1), :, :, :],
                                    ht[:, t, 1:17, SH, 1:33])
                # read sibling halo of t-1: a full conv1 t-phase (~90us)
                # after the matching write on the other side
                if t >= 1:
                    halo_read(t - 1)

            # ---- conv2 ----
            # runs: edge d=0 and d=15 alone (N=256, zero-pad taps skipped),
            # interior d as 7 pairs (N=512). Taps alternate between PE column
            # groups (psum partitions 0:64 / 64:128) so adjacent matmuls run
            # concurrently; halves summed via Scalar+DVE into the stage tile.
            runs = [(0, 1)] + [(d0, 2) for d0 in range(1, 15, 2)] + [(15, 1)]
            scrapd = dr.tile([64, 32], fp32, name="scrapd")
            for t in range(T):
                for d0, nd in runs:
                    taps = [(kt, ku, kv, kw) for kt in _t_taps(t)
                            for ku in range(3) if 0 < d0 + ku < 17 or nd == 2
                            for kv in range(3) for kw in range(3)]
                    nn = nd * SH * W
                    lo = taps[0::2]
                    hi = taps[1::2]
                    ps = p2.tile([128, N2], fp32)
                    for i in range(len(lo)):
                        for half, base, tp_pos in ((lo, 0, (0, 0)),
                                                   (hi, 64, (0, 64))):
                            if i >= len(half):
                                continue
                            kt, ku, kv, kw = half[i]
                            gi = _g81(kt, ku, kv, kw)
                            rhs = ht[:, t + kt - 1, d0 + ku:d0 + ku + nd,
                                     kv:kv + SH, kw:kw + W]
                            nc.tensor.matmul(
                                ps[base:base + 64, 0:nn], w2[:, gi, :], rhs,
                                start=(i == 0), stop=(i == len(half) - 1),
                                tile_position=tp_pos)
                    st = stp.tile([64, N2], fp32)
                    nc.scalar.activation(
                        st[:, 0:nn], ps[64:128, 0:nn],
                        mybir.ActivationFunctionType.Identity, bias=b2[:, 0:1])
                    nc.vector.tensor_add(st[:, 0:nn], st[:, 0:nn],
                                         ps[0:64, 0:nn])
                    nc.sync.dma_start(
                        y_d.ap()[:, t, d0 * SH * W:d0 * SH * W + nn],
                        st[:, 0:nn])
                if t == 0:
                    # gate the last halo readback on conv2 t=0 having
                    # drained, so it runs a conv2 t-phase after the
                    # sibling's t=3 write (conv2 t>=2 is what consumes it)
                    nc.gpsimd.dma_start(scrapd[:, :], st[:, 0:32])
                    halo_read(T - 1)
    nc.compile()
    return nc


def kernel(x, w1, b1, w2, b2):
    from concourse.bass_utils import run_bass_kernel_spmd

    hostd = _make_host_arrays(x, w1, b1, w2, b2)
    if "nc" not in _cache:
        _cache["nc"] = _build_module()
    nc = _cache["nc"]

    in_maps = _in_maps(hostd)

    def one_run():
        res = run_bass_kernel_spmd(nc, in_maps, core_ids=list(range(NCORES)))
        y = np.zeros((B, C_OUT, T, D, H, W), np.float32)
        for core in range(NCORES):
            b, j = divmod(core, NJ)
            yc = res.results[core]["y"].reshape(C_OUT, T, D, SH, W)
            if j % 2 == 0:
                y[b, :, :, :, SH * j:SH * (j + 1), :] = yc
            else:
                y[b, :, :, :, SH * j:SH * (j + 1), :] = yc[:, :, :, ::-1, :]
        return y

    # the cross-core halo exchange relies on launch-aligned cores; a rare
    # skewed launch corrupts a run, so require two consecutive identical
    # finite outputs (correct runs are bitwise deterministic)
    prev = one_run()
    for _ in range(4):
        cur = one_run()
        if np.isfinite(cur).all() and np.array_equal(cur, prev):
            return cur
        prev = cur
    return prev
